# revision 18
# baseline (speedup 1.0000x reference)
"""AdaptiveRankingLoss on 8 Trainium2 NeuronCores (Bass/Tile), upper-triangle v6.

Math
----
reference:  loss = sum_{i<j, |t_i-t_j|>=0.05} 0.5*(w_i+w_j)*relu(-sign(td)*pd + m) / count
            td = t_i - t_j, pd = p_i - p_j, m = ms*0.08*clip(|td|, 0.1, 1.0)

Every per-pair factor is symmetric in i<->j, so each unordered pair is computed
once.  The 64x64 grid of 128-row blocks is covered by a circulant schedule:
row-block I processes column-blocks J in the wrapped window [I, I+n_I) mod 64,
n_I = 33 for I<=31 and 32 for I>=32; every unordered block pair lands in
exactly one window, and the diagonal block leads each window.  Core k owns
blocks {4k..4k+3, 32+4k..32+4k+3}: identical shapes on every core.

Column data is laid out per-core ROTATED by 4k blocks, with the first 3 blocks
duplicated as a tail, so every window is one contiguous slice of a single
[128, 8576] broadcast tile per tensor.

v6 pipeline per block (rows on partitions, window cols on free), bf16.
Engine budget tuned to measured rates: DVE tensor_scalar runs 4x, tensor_tensor
2x, ACT 1x (scalar_tensor_tensor runs 1x on DVE - avoided entirely):
    ACT: ad   = Abs( tq_j - tq_i )        tq = bf16(0.08*ms*t)
    ACT: s    = Sign( tq_i - tq_j )
    DVE: v    = (ad is_ge theta)          theta = 0.004*ms
    DVE: pd   = pq_j - pq_i
    DVE: mg   = ad max lo                 lo = 0.008*ms; upper clip at 0.08*ms
                                          never binds for targets in [0,1)
    DVE: q    = pd * s
    DVE: vp   = q + mg
    ACT: viol[:, :XS]  = Relu(vp)         column-split to balance ACT vs DVE
    DVE: viol[:, XS:]  = vp max 0
    DVE: g    = viol * v
PE does ONE two-column transpose-reduce per 128-col chunk of g:
    ps_col[:, 2c:2c+2] = lhsT=g_chunk @ rhs=[ones, w_row]
giving the plain colsum AND the w_i-weighted colsum in one stationary pass;
sum of weighted colsums = S_r, so no separate row-reduce streams.  The pair
count C is computed host-side by a sorted two-pointer over raw targets.
The diagonal block is computed UNMASKED: its lower triangle equals its upper
triangle exactly (all factors symmetric, bf16 ops commute under sign flip) and
i==j dies via v (ad=0).  Host halves the diag accumulators / diag colsum chunk.

Host combines in f64:
    S_r = sum(Srd)/2 + sum(Srr);  C = sum(Cd)/2 + sum(Cr)
    S_c = sum_slot,c,p colsum[p, 33*slot+c] * w_col[J*128+p] (diag halved)
    loss = 0.5*(S_r + S_c) / C
All t/p/w values are bf16-quantized identically on host for row scalars and
column data so pairwise terms stay exactly symmetric.
"""

import sys

if "/opt/trn_rl_repo" not in sys.path:
    sys.path.insert(0, "/opt/trn_rl_repo")

import numpy as np
import ml_dtypes

N = 8192
P = 128
N_CORES = 8
NBLOCKS_TOTAL = N // P                 # 64 row blocks globally
SLOTS = 8                              # row blocks per core
LC = N + 3 * P                         # 8576 local (rotated) columns
XS = 3520                             # viol column split: [0,XS) ACT, [XS,L) DVE
# per-slot window start / length in the local column layout
SLOT_START = [P * i for i in range(4)] + [N // 2 + P * i for i in range(4)]
SLOT_LEN = [33 * P] * 4 + [32 * P] * 4

_CACHE = {}


def _core_blocks(core):
    return [4 * core + i for i in range(4)] + [32 + 4 * core + i for i in range(4)]


def _window(I):
    n = 33 if I <= 31 else 32
    return [(I + j) % NBLOCKS_TOTAL for j in range(n)]


def _mm_chunks(start, end):
    f = start
    while f < end:
        yield f, min(f + 512, end)
        f = min(f + 512, end)


def _build():
    from contextlib import ExitStack
    from concourse import bacc, tile, mybir

    BF16 = mybir.dt.bfloat16
    F32 = mybir.dt.float32
    Alu = mybir.AluOpType
    Act = mybir.ActivationFunctionType

    nc = bacc.Bacc("TRN2", target_bir_lowering=False, debug=False,
                   num_devices=N_CORES)

    tql_ext = nc.dram_tensor("tql", [P, LC], BF16, kind="ExternalInput").ap()
    pql_ext = nc.dram_tensor("pql", [P, LC], BF16, kind="ExternalInput").ap()
    # aux f32: 0:8 ti | 8:16 nti | 16:24 pi | 32 theta | 33 lo
    aux_ext = nc.dram_tensor("aux", [P, 36], F32, kind="ExternalInput").ap()
    # auxb bf16 [P,16]: col 2b = 1.0, col 2b+1 = row weights of slot b
    auxb_ext = nc.dram_tensor("auxb", [P, 16], BF16, kind="ExternalInput").ap()
    # out f32: [P, 528]: interleaved per chunk [colsum, w-weighted colsum];
    # slots 0-3 in 0:264, slots 4-7 in 264:528
    out_ext = nc.dram_tensor("out", [P, 528], F32, kind="ExternalOutput").ap()

    with tile.TileContext(nc) as tc:
        with ExitStack() as ctx:
            singles = ctx.enter_context(tc.tile_pool(name="singles", bufs=1))
            work = ctx.enter_context(tc.tile_pool(name="work", bufs=2))
            psum = ctx.enter_context(tc.tile_pool(name="psum", bufs=1, space="PSUM"))

            aux_sb = singles.tile([P, 36], F32)
            nc.sync.dma_start(out=aux_sb[:], in_=aux_ext[:])
            auxb_sb = singles.tile([P, 16], BF16)
            nc.sync.dma_start(out=auxb_sb[:], in_=auxb_ext[:])

            ones_sb = singles.tile([P, 1], BF16)
            nc.gpsimd.memset(ones_sb[:], 1.0)
            zerob_sb = singles.tile([P, 1], BF16)
            nc.gpsimd.memset(zerob_sb[:], 0.0)

            tqb = singles.tile([P, LC], BF16)
            pqb = singles.tile([P, LC], BF16)
            # column data arrives host-pre-broadcast: plain contiguous DMAs,
            # chunked for early compute start; tqb first (first dependency)
            BCH = LC // 8  # 1072
            for eng, dst, src_ in ((nc.sync, tqb, tql_ext),
                                   (nc.gpsimd, pqb, pql_ext)):
                for c0 in range(0, LC, BCH):
                    sl = slice(c0, c0 + BCH)
                    eng.dma_start(out=dst[:, sl], in_=src_[:, sl])

            # no memset needed: every ps_cola column is matmul-written
            # (33-chunk slots fill all 66 slots) and the drain pushers cover
            # ps_colb's unused columns
            ps_cola = psum.tile([P, 264], F32)
            ps_colb = psum.tile([P, 264], F32)

            def _ranges(b):
                # first/last slot split into halves: ramps DVE up earlier at
                # the start, overlaps PE column-reduce with DVE at the end
                L = SLOT_LEN[b]
                if b == 0:
                    qt = (L // 4 // P) * P
                    return [(0, qt), (qt, 2 * qt), (2 * qt, L)]
                if b == SLOTS - 1:
                    h = (L // 2 // P) * P
                    return [(0, h), (h, L)]
                return [(0, L)]

            def emit_adsign(b):
                st, L = SLOT_START[b], SLOT_LEN[b]
                ad = work.tile([P, L], BF16, tag="ad", bufs=3)
                s = work.tile([P, L], BF16, tag="s", bufs=3)
                for c0, c1 in _ranges(b):
                    nc.scalar.activation(out=ad[:, c0:c1],
                                         in_=tqb[:, st + c0:st + c1],
                                         func=Act.Abs,
                                         bias=aux_sb[:, 8 + b:9 + b], scale=1.0)
                    nc.scalar.activation(out=s[:, c0:c1],
                                         in_=tqb[:, st + c0:st + c1],
                                         func=Act.Sign,
                                         bias=aux_sb[:, b:b + 1], scale=-1.0)
                return ad, s

            pend = emit_adsign(0)
            last = {}
            for b in range(SLOTS):
                ad, s = pend
                st, L = SLOT_START[b], SLOT_LEN[b]
                nchunk = L // P
                v = work.tile([P, L], BF16, tag="v", bufs=2)
                pd = work.tile([P, L], BF16, tag="pd", bufs=2)
                mg = work.tile([P, L], BF16, tag="mg", bufs=2)
                q = work.tile([P, L], BF16, tag="q", bufs=2)
                vp = work.tile([P, L], BF16, tag="vp", bufs=2)
                viol = work.tile([P, L], BF16, tag="viol", bufs=2)
                g = work.tile([P, L], BF16, tag="g", bufs=2)
                for ri, (c0, c1) in enumerate(_ranges(b)):
                    r = slice(c0, c1)
                    nc.vector.tensor_scalar(
                        out=v[:, r], in0=ad[:, r], scalar1=aux_sb[:, 32:33],
                        scalar2=None, op0=Alu.is_ge)
                    nc.vector.tensor_scalar(
                        out=pd[:, r], in0=pqb[:, st + c0:st + c1],
                        scalar1=aux_sb[:, 16 + b:17 + b],
                        scalar2=None, op0=Alu.subtract)
                    nc.vector.tensor_scalar(
                        out=mg[:, r], in0=ad[:, r], scalar1=aux_sb[:, 33:34],
                        scalar2=None, op0=Alu.max)
                    nc.vector.tensor_tensor(out=q[:, r], in0=pd[:, r],
                                            in1=s[:, r], op=Alu.mult)
                    nc.vector.tensor_tensor(out=vp[:, r], in0=q[:, r],
                                            in1=mg[:, r], op=Alu.add)
                    if b + 1 < SLOTS and ri == len(_ranges(b)) - 1:
                        pend = emit_adsign(b + 1)
                    # relu, column-split across ACT and DVE for engine balance
                    a0, a1 = c0, min(c1, XS)
                    if a1 > a0:
                        nc.scalar.activation(out=viol[:, a0:a1],
                                             in_=vp[:, a0:a1], func=Act.Relu)
                    d0, d1 = max(c0, XS), c1
                    if d1 > d0:
                        nc.vector.tensor_scalar(
                            out=viol[:, d0:d1], in0=vp[:, d0:d1], scalar1=0.0,
                            scalar2=None, op0=Alu.max)
                    nc.vector.tensor_tensor(out=g[:, r], in0=viol[:, r],
                                            in1=v[:, r], op=Alu.mult)

                # per-chunk transpose-reduce with TWO rhs columns:
                # [colsum, w_i-weighted colsum] in one stationary pass.
                # S_r = sum of weighted colsums, so no row-reduce streams.
                ps_c = ps_cola if b < 4 else ps_colb
                o0 = 66 * (b % 4)
                for c in range(nchunk):
                    nc.tensor.matmul(
                        ps_c[:, o0 + 2 * c:o0 + 2 * c + 2],
                        lhsT=g[:, c * P:(c + 1) * P],
                        rhs=auxb_sb[:, 2 * b:2 * b + 2],
                        start=True, stop=True)
                last = {"g": g, "v": v}

            # drain pushers: push PE->PSUM writeback of the last real writes
            # through before the reads below (unused columns of slots w/ 32
            # chunks: local cols 64:66 of each 66-block)
            pushers = []
            for uc in (64, 130, 196, 262):
                # slots 4-7 use only 64 of their 66 ps_colb column slots;
                # ps_cola (33-chunk slots) has NO unused columns
                pushers.append(nc.tensor.matmul(
                    ps_colb[:, uc:uc + 2], lhsT=last["g"][:, 0:P],
                    rhs=auxb_sb[:, 0:2], start=True, stop=True))

            out_sb = singles.tile([P, 528], F32)
            c0 = nc.scalar.copy(out=out_sb[:, 0:264], in_=ps_cola[:])
            c1 = nc.scalar.copy(out=out_sb[:, 264:528], in_=ps_colb[:])
            for cc in (c0, c1):
                for pp in pushers:
                    tile.add_dep_helper(cc.ins, pp.ins,
                                        reason="final copy waits drain pushers")
            nc.sync.dma_start(out=out_ext[:], in_=out_sb[:])

    nc.compile()
    return nc


def _get_nc():
    if "nc" not in _CACHE:
        _CACHE["nc"] = _build()
    return _CACHE["nc"]


def _prepare_in_maps(predictions, targets, snr_weights, margin_scale):
    ms = float(margin_scale)
    bf16 = ml_dtypes.bfloat16

    t = np.asarray(targets, np.float32)
    p = np.asarray(predictions, np.float32)
    w = np.asarray(snr_weights, np.float32)

    # bf16-quantize once; identical values feed column data and row scalars so
    # every pairwise term is exactly symmetric.
    tq = (0.08 * ms * t).astype(bf16)
    pq = p.astype(bf16)
    wq = w.astype(bf16)
    tqf = tq.astype(np.float32)
    pqf = pq.astype(np.float32)

    in_maps = []
    for core in range(N_CORES):
        rot = 4 * core * P
        # rotated layout + 3-block tail so every window is contiguous
        idx = (rot + np.arange(LC)) % N
        tql = np.ascontiguousarray(
            np.broadcast_to(tq[idx].reshape(1, LC), (P, LC)))
        pql = np.ascontiguousarray(
            np.broadcast_to(pq[idx].reshape(1, LC), (P, LC)))
        blocks = _core_blocks(core)
        ti = np.empty((P, SLOTS), np.float32)
        pi = np.empty((P, SLOTS), np.float32)
        wib = np.empty((P, SLOTS), np.float32)
        for slot, I in enumerate(blocks):
            rows = slice(I * P, (I + 1) * P)
            ti[:, slot] = tqf[rows]
            pi[:, slot] = pqf[rows]
            wib[:, slot] = wq[rows]
        cst = np.zeros((P, 4), np.float32)
        cst[:, 0] = np.float32(0.05 * 0.08 * ms)   # theta
        cst[:, 1] = np.float32(0.1 * 0.08 * ms)    # lo
        aux = np.concatenate([ti, -ti, pi, -pi, cst], axis=1)
        auxw = np.empty((P, 2 * SLOTS), np.float32)
        auxw[:, 0::2] = 1.0
        auxw[:, 1::2] = wib
        in_maps.append({"tql": tql, "pql": pql,
                        "aux": aux.astype(np.float32),
                        "auxb": auxw.astype(bf16)})
    return in_maps


def _numpy_fallback(predictions, targets, snr_weights, margin_scale):
    t = np.asarray(targets, np.float64)
    p = np.asarray(predictions, np.float64)
    w = np.asarray(snr_weights, np.float64)
    ms = float(margin_scale)
    total = 0.0
    count = 0
    for i0 in range(0, N, 512):
        i1 = min(i0 + 512, N)
        td = t[i0:i1, None] - t[None, :]
        ad = np.abs(td)
        upper = (np.arange(i0, i1)[:, None] < np.arange(N)[None, :])
        valid = upper & (ad >= 0.05)
        margin = ms * 0.08 * np.clip(ad, 0.1, 1.0)
        pdm = p[i0:i1, None] - p[None, :]
        viol = np.maximum(-np.sign(td) * pdm + margin, 0.0)
        pw = 0.5 * (w[i0:i1, None] + w[None, :])
        total += float((pw * viol)[valid].sum())
        count += int(valid.sum())
    return np.float32(total / count if count > 0 else 0.0)


def kernel(predictions, targets, snr_weights, margin_scale):
    from concourse.bass_utils import run_bass_kernel_spmd

    if float(margin_scale) <= 0.0:
        return _numpy_fallback(predictions, targets, snr_weights, margin_scale)

    nc = _get_nc()
    in_maps = _prepare_in_maps(predictions, targets, snr_weights, margin_scale)
    res = run_bass_kernel_spmd(nc, in_maps, core_ids=list(range(N_CORES)))

    bf16 = ml_dtypes.bfloat16
    wqf = np.asarray(snr_weights, np.float32).astype(bf16).astype(np.float64)

    # count on host: pairs with t_i - t_j >= 0.05 via sorted two-pointer
    # (reference f32/f64 semantics; boundary mismatch vs the device's bf16
    # mask is ~1e-5 of pairs)
    ts_sorted = np.sort(np.asarray(targets, np.float64))
    C = float(np.searchsorted(ts_sorted, ts_sorted - 0.05, side="right").sum())

    S_r = 0.0
    S_c = 0.0
    for core in range(N_CORES):
        o = np.asarray(res.results[core]["out"], np.float64)
        blocks = _core_blocks(core)
        for slot, I in enumerate(blocks):
            base = 264 * (slot // 4) + 66 * (slot % 4)
            win = _window(I)
            for c, J in enumerate(win):
                w_col = wqf[J * P:(J + 1) * P]
                scale = 0.5 if c == 0 else 1.0
                S_c += scale * float(w_col @ o[:, base + 2 * c])
                S_r += scale * float(o[:, base + 2 * c + 1].sum())
    loss = 0.5 * (S_r + S_c) / C if C > 0 else 0.0
    return np.float32(loss)


# revision 19
# speedup vs baseline: 1.0137x; 1.0137x over previous
"""AdaptiveRankingLoss on 8 Trainium2 NeuronCores (Bass/Tile), upper-triangle v6.

Math
----
reference:  loss = sum_{i<j, |t_i-t_j|>=0.05} 0.5*(w_i+w_j)*relu(-sign(td)*pd + m) / count
            td = t_i - t_j, pd = p_i - p_j, m = ms*0.08*clip(|td|, 0.1, 1.0)

Every per-pair factor is symmetric in i<->j, so each unordered pair is computed
once.  The 64x64 grid of 128-row blocks is covered by a circulant schedule:
row-block I processes column-blocks J in the wrapped window [I, I+n_I) mod 64,
n_I = 33 for I<=31 and 32 for I>=32; every unordered block pair lands in
exactly one window, and the diagonal block leads each window.  Core k owns
blocks {4k..4k+3, 32+4k..32+4k+3}: identical shapes on every core.

Column data is laid out per-core ROTATED by 4k blocks, with the first 3 blocks
duplicated as a tail, so every window is one contiguous slice of a single
[128, 8576] broadcast tile per tensor.

v6 pipeline per block (rows on partitions, window cols on free), bf16.
Engine budget tuned to measured rates: DVE tensor_scalar runs 4x, tensor_tensor
2x, ACT 1x (scalar_tensor_tensor runs 1x on DVE - avoided entirely):
    ACT: ad   = Abs( tq_j - tq_i )        tq = bf16(0.08*ms*t)
    ACT: s    = Sign( tq_i - tq_j )
    DVE: v    = (ad is_ge theta)          theta = 0.004*ms
    DVE: pd   = pq_j - pq_i
    DVE: mg   = ad max lo                 lo = 0.008*ms; upper clip at 0.08*ms
                                          never binds for targets in [0,1)
    DVE: q    = pd * s
    DVE: vp   = q + mg
    ACT: viol[:, :XS]  = Relu(vp)         column-split to balance ACT vs DVE
    DVE: viol[:, XS:]  = vp max 0
    DVE: g    = viol * v
PE does ONE two-column transpose-reduce per 128-col chunk of g:
    ps_col[:, 2c:2c+2] = lhsT=g_chunk @ rhs=[ones, w_row]
giving the plain colsum AND the w_i-weighted colsum in one stationary pass;
sum of weighted colsums = S_r, so no separate row-reduce streams.  The pair
count C is computed host-side by a sorted two-pointer over raw targets.
The diagonal block is computed UNMASKED: its lower triangle equals its upper
triangle exactly (all factors symmetric, bf16 ops commute under sign flip) and
i==j dies via v (ad=0).  Host halves the diag accumulators / diag colsum chunk.

Host combines in f64:
    S_r = sum(Srd)/2 + sum(Srr);  C = sum(Cd)/2 + sum(Cr)
    S_c = sum_slot,c,p colsum[p, 33*slot+c] * w_col[J*128+p] (diag halved)
    loss = 0.5*(S_r + S_c) / C
All t/p/w values are bf16-quantized identically on host for row scalars and
column data so pairwise terms stay exactly symmetric.
"""

import sys

if "/opt/trn_rl_repo" not in sys.path:
    sys.path.insert(0, "/opt/trn_rl_repo")

import numpy as np
import ml_dtypes

N = 8192
P = 128
N_CORES = 8
NBLOCKS_TOTAL = N // P                 # 64 row blocks globally
SLOTS = 8                              # row blocks per core
LC = N + 3 * P                         # 8576 local (rotated) columns
XS = 3520                             # viol column split: [0,XS) ACT, [XS,L) DVE
# per-slot window start / length in the local column layout
SLOT_START = [P * i for i in range(4)] + [N // 2 + P * i for i in range(4)]
SLOT_LEN = [33 * P] * 4 + [32 * P] * 4

_CACHE = {}


def _core_blocks(core):
    return [4 * core + i for i in range(4)] + [32 + 4 * core + i for i in range(4)]


def _window(I):
    n = 33 if I <= 31 else 32
    return [(I + j) % NBLOCKS_TOTAL for j in range(n)]


def _mm_chunks(start, end):
    f = start
    while f < end:
        yield f, min(f + 512, end)
        f = min(f + 512, end)


def _build():
    from contextlib import ExitStack
    from concourse import bacc, tile, mybir

    BF16 = mybir.dt.bfloat16
    F32 = mybir.dt.float32
    Alu = mybir.AluOpType
    Act = mybir.ActivationFunctionType

    nc = bacc.Bacc("TRN2", target_bir_lowering=False, debug=False,
                   num_devices=N_CORES)

    tql_ext = nc.dram_tensor("tql", [P, LC], BF16, kind="ExternalInput").ap()
    pql_ext = nc.dram_tensor("pql", [P, LC], BF16, kind="ExternalInput").ap()
    # aux f32: 0:8 ti | 8:16 nti | 16:24 pi | 32 theta | 33 lo
    aux_ext = nc.dram_tensor("aux", [P, 36], F32, kind="ExternalInput").ap()
    # auxb bf16 [P,16]: col 2b = 1.0, col 2b+1 = row weights of slot b
    auxb_ext = nc.dram_tensor("auxb", [P, 16], BF16, kind="ExternalInput").ap()
    # out f32: [P, 528]: interleaved per chunk [colsum, w-weighted colsum];
    # slots 0-3 in 0:264, slots 4-7 in 264:528
    out_ext = nc.dram_tensor("out", [P, 528], F32, kind="ExternalOutput").ap()

    with tile.TileContext(nc) as tc:
        with ExitStack() as ctx:
            singles = ctx.enter_context(tc.tile_pool(name="singles", bufs=1))
            work = ctx.enter_context(tc.tile_pool(name="work", bufs=2))
            psum = ctx.enter_context(tc.tile_pool(name="psum", bufs=1, space="PSUM"))

            aux_sb = singles.tile([P, 36], F32)
            nc.scalar.dma_start(out=aux_sb[:], in_=aux_ext[:])
            auxb_sb = singles.tile([P, 16], BF16)
            nc.scalar.dma_start(out=auxb_sb[:], in_=auxb_ext[:])

            ones_sb = singles.tile([P, 1], BF16)
            nc.gpsimd.memset(ones_sb[:], 1.0)
            zerob_sb = singles.tile([P, 1], BF16)
            nc.gpsimd.memset(zerob_sb[:], 0.0)

            tqb = singles.tile([P, LC], BF16)
            pqb = singles.tile([P, LC], BF16)
            # column data arrives host-pre-broadcast: plain contiguous DMAs,
            # chunked for early compute start; tqb first (first dependency)
            BCH = LC // 8  # 1072
            for eng, dst, src_ in ((nc.sync, tqb, tql_ext),
                                   (nc.gpsimd, pqb, pql_ext)):
                for c0 in range(0, LC, BCH):
                    sl = slice(c0, c0 + BCH)
                    eng.dma_start(out=dst[:, sl], in_=src_[:, sl])

            # no memset needed: every ps_cola column is matmul-written
            # (33-chunk slots fill all 66 slots) and the drain pushers cover
            # ps_colb's unused columns
            ps_cola = psum.tile([P, 264], F32)
            ps_colb = psum.tile([P, 264], F32)

            def _ranges(b):
                # first/last slot split into halves: ramps DVE up earlier at
                # the start, overlaps PE column-reduce with DVE at the end
                L = SLOT_LEN[b]
                if b == 0:
                    qt = (L // 4 // P) * P
                    return [(0, qt), (qt, 2 * qt), (2 * qt, L)]
                if b == SLOTS - 1:
                    h = (L // 2 // P) * P
                    return [(0, h), (h, L)]
                return [(0, L)]

            def emit_adsign(b):
                st, L = SLOT_START[b], SLOT_LEN[b]
                ad = work.tile([P, L], BF16, tag="ad", bufs=3)
                s = work.tile([P, L], BF16, tag="s", bufs=3)
                for c0, c1 in _ranges(b):
                    nc.scalar.activation(out=ad[:, c0:c1],
                                         in_=tqb[:, st + c0:st + c1],
                                         func=Act.Abs,
                                         bias=aux_sb[:, 8 + b:9 + b], scale=1.0)
                    nc.scalar.activation(out=s[:, c0:c1],
                                         in_=tqb[:, st + c0:st + c1],
                                         func=Act.Sign,
                                         bias=aux_sb[:, b:b + 1], scale=-1.0)
                return ad, s

            pend = emit_adsign(0)
            last = {}
            for b in range(SLOTS):
                ad, s = pend
                st, L = SLOT_START[b], SLOT_LEN[b]
                nchunk = L // P
                v = work.tile([P, L], BF16, tag="v", bufs=2)
                pd = work.tile([P, L], BF16, tag="pd", bufs=2)
                mg = work.tile([P, L], BF16, tag="mg", bufs=2)
                q = work.tile([P, L], BF16, tag="q", bufs=2)
                vp = work.tile([P, L], BF16, tag="vp", bufs=2)
                viol = work.tile([P, L], BF16, tag="viol", bufs=2)
                g = work.tile([P, L], BF16, tag="g", bufs=2)
                for ri, (c0, c1) in enumerate(_ranges(b)):
                    r = slice(c0, c1)
                    nc.vector.tensor_scalar(
                        out=v[:, r], in0=ad[:, r], scalar1=aux_sb[:, 32:33],
                        scalar2=None, op0=Alu.is_ge)
                    nc.vector.tensor_scalar(
                        out=pd[:, r], in0=pqb[:, st + c0:st + c1],
                        scalar1=aux_sb[:, 16 + b:17 + b],
                        scalar2=None, op0=Alu.subtract)
                    nc.vector.tensor_scalar(
                        out=mg[:, r], in0=ad[:, r], scalar1=aux_sb[:, 33:34],
                        scalar2=None, op0=Alu.max)
                    nc.vector.tensor_tensor(out=q[:, r], in0=pd[:, r],
                                            in1=s[:, r], op=Alu.mult)
                    nc.vector.tensor_tensor(out=vp[:, r], in0=q[:, r],
                                            in1=mg[:, r], op=Alu.add)
                    if b + 1 < SLOTS and ri == len(_ranges(b)) - 1:
                        pend = emit_adsign(b + 1)
                    # relu, column-split across ACT and DVE for engine balance
                    a0, a1 = c0, min(c1, XS)
                    if a1 > a0:
                        nc.scalar.activation(out=viol[:, a0:a1],
                                             in_=vp[:, a0:a1], func=Act.Relu)
                    d0, d1 = max(c0, XS), c1
                    if d1 > d0:
                        nc.vector.tensor_scalar(
                            out=viol[:, d0:d1], in0=vp[:, d0:d1], scalar1=0.0,
                            scalar2=None, op0=Alu.max)
                    nc.vector.tensor_tensor(out=g[:, r], in0=viol[:, r],
                                            in1=v[:, r], op=Alu.mult)

                # per-chunk transpose-reduce with TWO rhs columns:
                # [colsum, w_i-weighted colsum] in one stationary pass.
                # S_r = sum of weighted colsums, so no row-reduce streams.
                ps_c = ps_cola if b < 4 else ps_colb
                o0 = 66 * (b % 4)
                for c in range(nchunk):
                    nc.tensor.matmul(
                        ps_c[:, o0 + 2 * c:o0 + 2 * c + 2],
                        lhsT=g[:, c * P:(c + 1) * P],
                        rhs=auxb_sb[:, 2 * b:2 * b + 2],
                        start=True, stop=True)
                last = {"g": g, "v": v}

            # drain pushers: push PE->PSUM writeback of the last real writes
            # through before the reads below (unused columns of slots w/ 32
            # chunks: local cols 64:66 of each 66-block)
            pushers = []
            for uc in (64, 130, 196, 262):
                # slots 4-7 use only 64 of their 66 ps_colb column slots;
                # ps_cola (33-chunk slots) has NO unused columns
                pushers.append(nc.tensor.matmul(
                    ps_colb[:, uc:uc + 2], lhsT=last["g"][:, 0:P],
                    rhs=auxb_sb[:, 0:2], start=True, stop=True))

            out_sb = singles.tile([P, 528], F32)
            c0 = nc.scalar.copy(out=out_sb[:, 0:264], in_=ps_cola[:])
            c1 = nc.scalar.copy(out=out_sb[:, 264:528], in_=ps_colb[:])
            for cc in (c0, c1):
                for pp in pushers:
                    tile.add_dep_helper(cc.ins, pp.ins,
                                        reason="final copy waits drain pushers")
            nc.sync.dma_start(out=out_ext[:], in_=out_sb[:])

    nc.compile()
    return nc


def _get_nc():
    if "nc" not in _CACHE:
        _CACHE["nc"] = _build()
    return _CACHE["nc"]


def _prepare_in_maps(predictions, targets, snr_weights, margin_scale):
    ms = float(margin_scale)
    bf16 = ml_dtypes.bfloat16

    t = np.asarray(targets, np.float32)
    p = np.asarray(predictions, np.float32)
    w = np.asarray(snr_weights, np.float32)

    # bf16-quantize once; identical values feed column data and row scalars so
    # every pairwise term is exactly symmetric.
    tq = (0.08 * ms * t).astype(bf16)
    pq = p.astype(bf16)
    wq = w.astype(bf16)
    tqf = tq.astype(np.float32)
    pqf = pq.astype(np.float32)

    in_maps = []
    for core in range(N_CORES):
        rot = 4 * core * P
        # rotated layout + 3-block tail so every window is contiguous
        idx = (rot + np.arange(LC)) % N
        tql = np.ascontiguousarray(
            np.broadcast_to(tq[idx].reshape(1, LC), (P, LC)))
        pql = np.ascontiguousarray(
            np.broadcast_to(pq[idx].reshape(1, LC), (P, LC)))
        blocks = _core_blocks(core)
        ti = np.empty((P, SLOTS), np.float32)
        pi = np.empty((P, SLOTS), np.float32)
        wib = np.empty((P, SLOTS), np.float32)
        for slot, I in enumerate(blocks):
            rows = slice(I * P, (I + 1) * P)
            ti[:, slot] = tqf[rows]
            pi[:, slot] = pqf[rows]
            wib[:, slot] = wq[rows]
        cst = np.zeros((P, 4), np.float32)
        cst[:, 0] = np.float32(0.05 * 0.08 * ms)   # theta
        cst[:, 1] = np.float32(0.1 * 0.08 * ms)    # lo
        aux = np.concatenate([ti, -ti, pi, -pi, cst], axis=1)
        auxw = np.empty((P, 2 * SLOTS), np.float32)
        auxw[:, 0::2] = 1.0
        auxw[:, 1::2] = wib
        in_maps.append({"tql": tql, "pql": pql,
                        "aux": aux.astype(np.float32),
                        "auxb": auxw.astype(bf16)})
    return in_maps


def _numpy_fallback(predictions, targets, snr_weights, margin_scale):
    t = np.asarray(targets, np.float64)
    p = np.asarray(predictions, np.float64)
    w = np.asarray(snr_weights, np.float64)
    ms = float(margin_scale)
    total = 0.0
    count = 0
    for i0 in range(0, N, 512):
        i1 = min(i0 + 512, N)
        td = t[i0:i1, None] - t[None, :]
        ad = np.abs(td)
        upper = (np.arange(i0, i1)[:, None] < np.arange(N)[None, :])
        valid = upper & (ad >= 0.05)
        margin = ms * 0.08 * np.clip(ad, 0.1, 1.0)
        pdm = p[i0:i1, None] - p[None, :]
        viol = np.maximum(-np.sign(td) * pdm + margin, 0.0)
        pw = 0.5 * (w[i0:i1, None] + w[None, :])
        total += float((pw * viol)[valid].sum())
        count += int(valid.sum())
    return np.float32(total / count if count > 0 else 0.0)


def kernel(predictions, targets, snr_weights, margin_scale):
    from concourse.bass_utils import run_bass_kernel_spmd

    if float(margin_scale) <= 0.0:
        return _numpy_fallback(predictions, targets, snr_weights, margin_scale)

    nc = _get_nc()
    in_maps = _prepare_in_maps(predictions, targets, snr_weights, margin_scale)
    res = run_bass_kernel_spmd(nc, in_maps, core_ids=list(range(N_CORES)))

    bf16 = ml_dtypes.bfloat16
    wqf = np.asarray(snr_weights, np.float32).astype(bf16).astype(np.float64)

    # count on host: pairs with t_i - t_j >= 0.05 via sorted two-pointer
    # (reference f32/f64 semantics; boundary mismatch vs the device's bf16
    # mask is ~1e-5 of pairs)
    ts_sorted = np.sort(np.asarray(targets, np.float64))
    C = float(np.searchsorted(ts_sorted, ts_sorted - 0.05, side="right").sum())

    S_r = 0.0
    S_c = 0.0
    for core in range(N_CORES):
        o = np.asarray(res.results[core]["out"], np.float64)
        blocks = _core_blocks(core)
        for slot, I in enumerate(blocks):
            base = 264 * (slot // 4) + 66 * (slot % 4)
            win = _window(I)
            for c, J in enumerate(win):
                w_col = wqf[J * P:(J + 1) * P]
                scale = 0.5 if c == 0 else 1.0
                S_c += scale * float(w_col @ o[:, base + 2 * c])
                S_r += scale * float(o[:, base + 2 * c + 1].sum())
    loss = 0.5 * (S_r + S_c) / C if C > 0 else 0.0
    return np.float32(loss)


# revision 20
# speedup vs baseline: 1.0260x; 1.0121x over previous
"""AdaptiveRankingLoss on 8 Trainium2 NeuronCores (Bass/Tile), upper-triangle v6.

Math
----
reference:  loss = sum_{i<j, |t_i-t_j|>=0.05} 0.5*(w_i+w_j)*relu(-sign(td)*pd + m) / count
            td = t_i - t_j, pd = p_i - p_j, m = ms*0.08*clip(|td|, 0.1, 1.0)

Every per-pair factor is symmetric in i<->j, so each unordered pair is computed
once.  The 64x64 grid of 128-row blocks is covered by a circulant schedule:
row-block I processes column-blocks J in the wrapped window [I, I+n_I) mod 64,
n_I = 33 for I<=31 and 32 for I>=32; every unordered block pair lands in
exactly one window, and the diagonal block leads each window.  Core k owns
blocks {4k..4k+3, 32+4k..32+4k+3}: identical shapes on every core.

Column data is laid out per-core ROTATED by 4k blocks, with the first 3 blocks
duplicated as a tail, so every window is one contiguous slice of a single
[128, 8576] broadcast tile per tensor.

v6 pipeline per block (rows on partitions, window cols on free), bf16.
Engine budget tuned to measured rates: DVE tensor_scalar runs 4x, tensor_tensor
2x, ACT 1x (scalar_tensor_tensor runs 1x on DVE - avoided entirely):
    ACT: ad   = Abs( tq_j - tq_i )        tq = bf16(0.08*ms*t)
    ACT: s    = Sign( tq_i - tq_j )
    DVE: v    = (ad is_ge theta)          theta = 0.004*ms
    DVE: pd   = pq_j - pq_i
    DVE: mg   = ad max lo                 lo = 0.008*ms; upper clip at 0.08*ms
                                          never binds for targets in [0,1)
    DVE: q    = pd * s
    DVE: vp   = q + mg
    ACT: viol[:, :XS]  = Relu(vp)         column-split to balance ACT vs DVE
    DVE: viol[:, XS:]  = vp max 0
    DVE: g    = viol * v
PE does ONE two-column transpose-reduce per 128-col chunk of g:
    ps_col[:, 2c:2c+2] = lhsT=g_chunk @ rhs=[ones, w_row]
giving the plain colsum AND the w_i-weighted colsum in one stationary pass;
sum of weighted colsums = S_r, so no separate row-reduce streams.  The pair
count C is computed host-side by a sorted two-pointer over raw targets.
The diagonal block is computed UNMASKED: its lower triangle equals its upper
triangle exactly (all factors symmetric, bf16 ops commute under sign flip) and
i==j dies via v (ad=0).  Host halves the diag accumulators / diag colsum chunk.

Host combines in f64:
    S_r = sum(Srd)/2 + sum(Srr);  C = sum(Cd)/2 + sum(Cr)
    S_c = sum_slot,c,p colsum[p, 33*slot+c] * w_col[J*128+p] (diag halved)
    loss = 0.5*(S_r + S_c) / C
All t/p/w values are bf16-quantized identically on host for row scalars and
column data so pairwise terms stay exactly symmetric.
"""

import sys

if "/opt/trn_rl_repo" not in sys.path:
    sys.path.insert(0, "/opt/trn_rl_repo")

import numpy as np
import ml_dtypes

N = 8192
P = 128
N_CORES = 8
NBLOCKS_TOTAL = N // P                 # 64 row blocks globally
SLOTS = 8                              # row blocks per core
LC = N + 3 * P                         # 8576 local (rotated) columns
XS = 3520                             # viol column split: [0,XS) ACT, [XS,L) DVE
# per-slot window start / length in the local column layout
SLOT_START = [P * i for i in range(4)] + [N // 2 + P * i for i in range(4)]
SLOT_LEN = [33 * P] * 4 + [32 * P] * 4

_CACHE = {}


def _core_blocks(core):
    return [4 * core + i for i in range(4)] + [32 + 4 * core + i for i in range(4)]


def _window(I):
    n = 33 if I <= 31 else 32
    return [(I + j) % NBLOCKS_TOTAL for j in range(n)]


def _mm_chunks(start, end):
    f = start
    while f < end:
        yield f, min(f + 512, end)
        f = min(f + 512, end)


def _build():
    from contextlib import ExitStack
    from concourse import bacc, tile, mybir

    BF16 = mybir.dt.bfloat16
    F32 = mybir.dt.float32
    Alu = mybir.AluOpType
    Act = mybir.ActivationFunctionType

    nc = bacc.Bacc("TRN2", target_bir_lowering=False, debug=False,
                   num_devices=N_CORES)

    tql_ext = nc.dram_tensor("tql", [P, LC], BF16, kind="ExternalInput").ap()
    pql_ext = nc.dram_tensor("pql", [P, LC], BF16, kind="ExternalInput").ap()
    # aux f32: 0:8 ti | 8:16 nti | 16:24 pi | 32 theta | 33 lo
    aux_ext = nc.dram_tensor("aux", [P, 36], F32, kind="ExternalInput").ap()
    # auxb bf16 [P,16]: col 2b = 1.0, col 2b+1 = row weights of slot b
    auxb_ext = nc.dram_tensor("auxb", [P, 16], BF16, kind="ExternalInput").ap()
    # out f32: [P, 528]: interleaved per chunk [colsum, w-weighted colsum];
    # slots 0-3 in 0:264, slots 4-7 in 264:528
    out_ext = nc.dram_tensor("out", [P, 528], F32, kind="ExternalOutput").ap()

    with tile.TileContext(nc) as tc:
        with ExitStack() as ctx:
            singles = ctx.enter_context(tc.tile_pool(name="singles", bufs=1))
            work = ctx.enter_context(tc.tile_pool(name="work", bufs=2))
            psum = ctx.enter_context(tc.tile_pool(name="psum", bufs=1, space="PSUM"))

            aux_sb = singles.tile([P, 36], F32)
            nc.scalar.dma_start(out=aux_sb[:], in_=aux_ext[:])
            auxb_sb = singles.tile([P, 16], BF16)
            nc.scalar.dma_start(out=auxb_sb[:], in_=auxb_ext[:])

            ones_sb = singles.tile([P, 1], BF16)
            nc.gpsimd.memset(ones_sb[:], 1.0)
            zerob_sb = singles.tile([P, 1], BF16)
            nc.gpsimd.memset(zerob_sb[:], 0.0)

            tqb = singles.tile([P, LC], BF16)
            pqb = singles.tile([P, LC], BF16)
            # column data arrives host-pre-broadcast: plain contiguous DMAs,
            # chunked for early compute start; tqb first (first dependency)
            BCH = LC // 8  # 1072
            for eng, dst, src_ in ((nc.sync, tqb, tql_ext),
                                   (nc.gpsimd, pqb, pql_ext)):
                for c0 in range(0, LC, BCH):
                    sl = slice(c0, c0 + BCH)
                    eng.dma_start(out=dst[:, sl], in_=src_[:, sl])

            ps_cola = psum.tile([P, 264], F32)
            ps_colb = psum.tile([P, 264], F32)
            nc.vector.memset(ps_cola[:], 0.0)
            nc.vector.memset(ps_colb[:], 0.0)

            def _ranges(b):
                # first/last slot split into halves: ramps DVE up earlier at
                # the start, overlaps PE column-reduce with DVE at the end
                L = SLOT_LEN[b]
                if b == 0:
                    qt = (L // 4 // P) * P
                    return [(0, qt), (qt, 2 * qt), (2 * qt, L)]
                if b == SLOTS - 1:
                    h = (L // 2 // P) * P
                    return [(0, h), (h, L)]
                return [(0, L)]

            def emit_adsign(b):
                st, L = SLOT_START[b], SLOT_LEN[b]
                ad = work.tile([P, L], BF16, tag="ad", bufs=3)
                s = work.tile([P, L], BF16, tag="s", bufs=3)
                for c0, c1 in _ranges(b):
                    nc.scalar.activation(out=ad[:, c0:c1],
                                         in_=tqb[:, st + c0:st + c1],
                                         func=Act.Abs,
                                         bias=aux_sb[:, 8 + b:9 + b], scale=1.0)
                    nc.scalar.activation(out=s[:, c0:c1],
                                         in_=tqb[:, st + c0:st + c1],
                                         func=Act.Sign,
                                         bias=aux_sb[:, b:b + 1], scale=-1.0)
                return ad, s

            pend = emit_adsign(0)
            last = {}
            for b in range(SLOTS):
                ad, s = pend
                st, L = SLOT_START[b], SLOT_LEN[b]
                nchunk = L // P
                v = work.tile([P, L], BF16, tag="v", bufs=2)
                pd = work.tile([P, L], BF16, tag="pd", bufs=2)
                mg = work.tile([P, L], BF16, tag="mg", bufs=2)
                q = work.tile([P, L], BF16, tag="q", bufs=2)
                vp = work.tile([P, L], BF16, tag="vp", bufs=2)
                viol = work.tile([P, L], BF16, tag="viol", bufs=2)
                g = work.tile([P, L], BF16, tag="g", bufs=2)
                for ri, (c0, c1) in enumerate(_ranges(b)):
                    r = slice(c0, c1)
                    nc.vector.tensor_scalar(
                        out=v[:, r], in0=ad[:, r], scalar1=aux_sb[:, 32:33],
                        scalar2=None, op0=Alu.is_ge)
                    nc.vector.tensor_scalar(
                        out=pd[:, r], in0=pqb[:, st + c0:st + c1],
                        scalar1=aux_sb[:, 16 + b:17 + b],
                        scalar2=None, op0=Alu.subtract)
                    nc.vector.tensor_scalar(
                        out=mg[:, r], in0=ad[:, r], scalar1=aux_sb[:, 33:34],
                        scalar2=None, op0=Alu.max)
                    nc.vector.tensor_tensor(out=q[:, r], in0=pd[:, r],
                                            in1=s[:, r], op=Alu.mult)
                    nc.vector.tensor_tensor(out=vp[:, r], in0=q[:, r],
                                            in1=mg[:, r], op=Alu.add)
                    if b + 1 < SLOTS and ri == len(_ranges(b)) - 1:
                        pend = emit_adsign(b + 1)
                    # relu, column-split across ACT and DVE for engine balance
                    a0, a1 = c0, min(c1, XS)
                    if a1 > a0:
                        nc.scalar.activation(out=viol[:, a0:a1],
                                             in_=vp[:, a0:a1], func=Act.Relu)
                    d0, d1 = max(c0, XS), c1
                    if d1 > d0:
                        nc.vector.tensor_scalar(
                            out=viol[:, d0:d1], in0=vp[:, d0:d1], scalar1=0.0,
                            scalar2=None, op0=Alu.max)
                    nc.vector.tensor_tensor(out=g[:, r], in0=viol[:, r],
                                            in1=v[:, r], op=Alu.mult)

                # per-chunk transpose-reduce with TWO rhs columns:
                # [colsum, w_i-weighted colsum] in one stationary pass.
                # S_r = sum of weighted colsums, so no row-reduce streams.
                ps_c = ps_cola if b < 4 else ps_colb
                o0 = 66 * (b % 4)
                for c in range(nchunk):
                    nc.tensor.matmul(
                        ps_c[:, o0 + 2 * c:o0 + 2 * c + 2],
                        lhsT=g[:, c * P:(c + 1) * P],
                        rhs=auxb_sb[:, 2 * b:2 * b + 2],
                        start=True, stop=True)
                last = {"g": g, "v": v}

            # drain pushers: push PE->PSUM writeback of the last real writes
            # through before the reads below (unused columns of slots w/ 32
            # chunks: local cols 64:66 of each 66-block)
            pushers = []
            for uc in (64, 130, 196, 262):
                # slots 4-7 use only 64 of their 66 ps_colb column slots;
                # ps_cola (33-chunk slots) has NO unused columns
                pushers.append(nc.tensor.matmul(
                    ps_colb[:, uc:uc + 2], lhsT=last["g"][:, 0:P],
                    rhs=auxb_sb[:, 0:2], start=True, stop=True))

            out_sb = singles.tile([P, 528], F32)
            c0 = nc.scalar.copy(out=out_sb[:, 0:264], in_=ps_cola[:])
            c1 = nc.scalar.copy(out=out_sb[:, 264:528], in_=ps_colb[:])
            for cc in (c0, c1):
                for pp in pushers:
                    tile.add_dep_helper(cc.ins, pp.ins,
                                        reason="final copy waits drain pushers")
            nc.sync.dma_start(out=out_ext[:], in_=out_sb[:])

    nc.compile()
    return nc


def _get_nc():
    if "nc" not in _CACHE:
        _CACHE["nc"] = _build()
    return _CACHE["nc"]


def _prepare_in_maps(predictions, targets, snr_weights, margin_scale):
    ms = float(margin_scale)
    bf16 = ml_dtypes.bfloat16

    t = np.asarray(targets, np.float32)
    p = np.asarray(predictions, np.float32)
    w = np.asarray(snr_weights, np.float32)

    # bf16-quantize once; identical values feed column data and row scalars so
    # every pairwise term is exactly symmetric.
    tq = (0.08 * ms * t).astype(bf16)
    pq = p.astype(bf16)
    wq = w.astype(bf16)
    tqf = tq.astype(np.float32)
    pqf = pq.astype(np.float32)

    in_maps = []
    for core in range(N_CORES):
        rot = 4 * core * P
        # rotated layout + 3-block tail so every window is contiguous
        idx = (rot + np.arange(LC)) % N
        tql = np.ascontiguousarray(
            np.broadcast_to(tq[idx].reshape(1, LC), (P, LC)))
        pql = np.ascontiguousarray(
            np.broadcast_to(pq[idx].reshape(1, LC), (P, LC)))
        blocks = _core_blocks(core)
        ti = np.empty((P, SLOTS), np.float32)
        pi = np.empty((P, SLOTS), np.float32)
        wib = np.empty((P, SLOTS), np.float32)
        for slot, I in enumerate(blocks):
            rows = slice(I * P, (I + 1) * P)
            ti[:, slot] = tqf[rows]
            pi[:, slot] = pqf[rows]
            wib[:, slot] = wq[rows]
        cst = np.zeros((P, 4), np.float32)
        cst[:, 0] = np.float32(0.05 * 0.08 * ms)   # theta
        cst[:, 1] = np.float32(0.1 * 0.08 * ms)    # lo
        aux = np.concatenate([ti, -ti, pi, -pi, cst], axis=1)
        auxw = np.empty((P, 2 * SLOTS), np.float32)
        auxw[:, 0::2] = 1.0
        auxw[:, 1::2] = wib
        in_maps.append({"tql": tql, "pql": pql,
                        "aux": aux.astype(np.float32),
                        "auxb": auxw.astype(bf16)})
    return in_maps


def _numpy_fallback(predictions, targets, snr_weights, margin_scale):
    t = np.asarray(targets, np.float64)
    p = np.asarray(predictions, np.float64)
    w = np.asarray(snr_weights, np.float64)
    ms = float(margin_scale)
    total = 0.0
    count = 0
    for i0 in range(0, N, 512):
        i1 = min(i0 + 512, N)
        td = t[i0:i1, None] - t[None, :]
        ad = np.abs(td)
        upper = (np.arange(i0, i1)[:, None] < np.arange(N)[None, :])
        valid = upper & (ad >= 0.05)
        margin = ms * 0.08 * np.clip(ad, 0.1, 1.0)
        pdm = p[i0:i1, None] - p[None, :]
        viol = np.maximum(-np.sign(td) * pdm + margin, 0.0)
        pw = 0.5 * (w[i0:i1, None] + w[None, :])
        total += float((pw * viol)[valid].sum())
        count += int(valid.sum())
    return np.float32(total / count if count > 0 else 0.0)


def kernel(predictions, targets, snr_weights, margin_scale):
    from concourse.bass_utils import run_bass_kernel_spmd

    if float(margin_scale) <= 0.0:
        return _numpy_fallback(predictions, targets, snr_weights, margin_scale)

    nc = _get_nc()
    in_maps = _prepare_in_maps(predictions, targets, snr_weights, margin_scale)
    res = run_bass_kernel_spmd(nc, in_maps, core_ids=list(range(N_CORES)))

    bf16 = ml_dtypes.bfloat16
    wqf = np.asarray(snr_weights, np.float32).astype(bf16).astype(np.float64)

    # count on host: pairs with t_i - t_j >= 0.05 via sorted two-pointer
    # (reference f32/f64 semantics; boundary mismatch vs the device's bf16
    # mask is ~1e-5 of pairs)
    ts_sorted = np.sort(np.asarray(targets, np.float64))
    C = float(np.searchsorted(ts_sorted, ts_sorted - 0.05, side="right").sum())

    S_r = 0.0
    S_c = 0.0
    for core in range(N_CORES):
        o = np.asarray(res.results[core]["out"], np.float64)
        blocks = _core_blocks(core)
        for slot, I in enumerate(blocks):
            base = 264 * (slot // 4) + 66 * (slot % 4)
            win = _window(I)
            for c, J in enumerate(win):
                w_col = wqf[J * P:(J + 1) * P]
                scale = 0.5 if c == 0 else 1.0
                S_c += scale * float(w_col @ o[:, base + 2 * c])
                S_r += scale * float(o[:, base + 2 * c + 1].sum())
    loss = 0.5 * (S_r + S_c) / C if C > 0 else 0.0
    return np.float32(loss)
